# revision 19
# baseline (speedup 1.0000x reference)
"""Trainium2 Bass kernel for nn_ExtractModel (edit-distance vocab extraction).

Strategy (8 NeuronCores, data-parallel over batch, 2 rows/core):
  - Front-end (embedding matmul, conv1d, log-softmaxes, pos_lp) on PE/ACT/DVE.
  - sub tensor built on-chip via one-hot gather matmuls (PE).
  - Edit-distance DP in "g-space" (g = f + C*(i+j)): insert/delete become free,
    g is nonneg + monotone, and the whole inner recurrence
        g[i][j] = max(A[i][j], g[i-1][j], g[i][j-1])
    collapses into ONE tensor_tensor_scan per row; the adds A = g_shifted + s'
    run on DVE (SBUF) or PE (PSUM identity matmuls), per-tile switchable.
    Vocab words live on partitions, sorted by length into uniform tiles (group
    leftovers packed into 32-aligned runs of mixed tiles); (batch,start,j) on
    the free dim, trimmed to each row's viable start range (rows paired
    long+short across cores).  Cross-word scan contamination is neutralized by
    a per-(b,s) additive offset K*m (K > max achievable g), subtracted exactly
    at extraction.
  - top-8 per partition via max8/max_index, final argmax + exact float64
    re-scoring of the top candidates on host (guards the ~1e-3 offset rounding).
"""
import os
import numpy as np

# ---- problem constants (hardcoded; kernel.py must be self-contained) ----
B, L = 16, 50
LU, KU, DIM = 60, 100, 60
V, J = 2000, 10
MIN_WL, MAX_WL = 4, 10
NE = MAX_WL - MIN_WL + 1          # 7
C = 3.5                           # INS_DEL_COST
NEG = -9999.9
LOGU = float(np.log(0.01))        # UNEXTRACTED_PROB log
CW = 0.1                          # context weight
TWOC = 2.0 * C
K_OFF = 72.0                      # per-m offset; > max g (10 * 2C = 70)
BIGNEG = -1.0e6
N_CORES = 8
BPC = B // N_CORES                # 2 batch rows per core
SPOS = L + MAX_WL - 1             # 59 text positions incl. DP lookahead
TP = 128                          # partitions per vocab tile
PE_FRAC = float(os.environ.get("KERNEL_PE_FRAC", "0.0"))

_CACHE = {}


# --------------------------------------------------------------------------
# host-side meta
# --------------------------------------------------------------------------
def _vocab_meta(vocab_lengths):
    """Sort vocab by length; full 128-word tiles per length group; pack the
    leftovers into mixed tiles as 32-aligned partition runs.
    Tile = (wt-1, runs); run = (p0, n, l, base)."""
    order = np.argsort(vocab_lengths, kind="stable").astype(np.int64)
    vls = vocab_lengths[order].astype(np.int64)
    groups = []                    # (l, base, n)
    i0 = 0
    while i0 < V:
        l = int(vls[i0])
        g1 = i0
        while g1 < V and vls[g1] == l:
            g1 += 1
        groups.append((l, i0, g1 - i0))
        i0 = g1
    tiles = []
    leftovers = []
    for (l, base, n) in groups:
        nfull = n // TP
        for k in range(nfull):
            tiles.append((l, [(0, TP, l, base + k * TP)]))
        rem = n - nfull * TP
        if rem:
            leftovers.append((l, base + nfull * TP, rem))
    # pack leftovers (desc by l) into mixed tiles of 4 x 32-aligned blocks
    leftovers.sort(key=lambda x: -x[0])
    cur_runs, cur_blocks = [], 0
    for (l, base, n) in leftovers:
        nb = -(-n // 32)
        if cur_blocks + nb > 4:
            tiles.append((max(r[2] for r in cur_runs), cur_runs))
            cur_runs, cur_blocks = [], 0
        cur_runs.append((cur_blocks * 32, n, l, base))
        cur_blocks += nb
    if cur_runs:
        tiles.append((max(r[2] for r in cur_runs), cur_runs))
    return order, vls, tiles


def _pair_batches(lengths):
    """Pair longest with shortest batch rows; per-b-slot start-range maxima."""
    desc = np.argsort(-lengths, kind="stable")
    pairs = [(int(desc[c]), int(desc[B - 1 - c])) for c in range(N_CORES)]
    mh0 = max(int(lengths[p[0]]) - (MIN_WL - 1) for p in pairs)
    mh1 = max(int(lengths[p[1]]) - (MIN_WL - 1) for p in pairs)
    return pairs, (mh0, mh1)


def _branges(p0, p1):
    """Split [p0, p1) into engine-legal partition ranges (buddy-aligned)."""
    out = []
    while p0 < p1:
        for s_ in (128, 64, 32):
            if p0 % s_ == 0 and p0 + s_ <= p1:
                out.append((p0, p0 + s_))
                p0 += s_
                break
    return out


def _slab_plan(l, mh):
    """PE-mode PSUM slabs: m-chunks <=512 cols, within one b-block."""
    wt = l + 1
    slabs = []
    off = 0
    for w in mh:
        nch = -(-(w * wt) // 512)
        bnd = [off + (w * k) // nch for k in range(nch + 1)]
        slabs.extend((bnd[k], bnd[k + 1]) for k in range(nch))
        off += w
    return slabs


# --------------------------------------------------------------------------
# device program (SPMD; per-core data arrives via input tensors)
# --------------------------------------------------------------------------
def _build_program(tiles, mh):
    import concourse.bacc as bacc
    import concourse.mybir as mybir
    from concourse import tile as tl

    f32 = mybir.dt.float32
    ui32 = mybir.dt.uint32
    AX = mybir.AxisListType
    OP = mybir.AluOpType
    AF = mybir.ActivationFunctionType

    NT = len(tiles)
    NOH = sum(l for l, _ in tiles)
    M = sum(mh)
    GW = sum(1 + M * (l + 1) for l, _ in tiles)

    # PE-mode vs DVE-mode tiles
    tot_cols = sum(M * (l + 1) for l, _ in tiles)
    pe_mode = []
    cum = 0
    for (l, _) in tiles:
        pe_mode.append(cum < PE_FRAC * tot_cols)
        cum += M * (l + 1)

    nc = bacc.Bacc(None, target_bir_lowering=False)

    d_alT = nc.dram_tensor("alT", (KU, LU), f32, kind="ExternalInput")
    d_ur = nc.dram_tensor("ur", (KU, DIM), f32, kind="ExternalInput")
    d_urT = nc.dram_tensor("urT", (DIM, KU), f32, kind="ExternalInput")
    d_cwT = nc.dram_tensor("cwT", (DIM, 3, DIM), f32, kind="ExternalInput")
    d_cb = nc.dram_tensor("cb", (DIM, 1), f32, kind="ExternalInput")
    d_c2 = nc.dram_tensor("c2", (1, KU), f32, kind="ExternalInput")
    d_ohu = nc.dram_tensor("ohu", (LU, BPC, L + 2), f32, kind="ExternalInput")
    d_ohu2 = nc.dram_tensor("ohu2", (LU + 1, BPC * L), f32,
                            kind="ExternalInput")
    d_ohv = nc.dram_tensor("ohv", (KU, NOH * TP), f32, kind="ExternalInput")
    d_ident = nc.dram_tensor("ident", (128, 128), f32, kind="ExternalInput")
    d_cst = nc.dram_tensor("cst", (128, M * NE), f32, kind="ExternalInput")
    d_pscal = nc.dram_tensor("pscal", (128, NT), f32, kind="ExternalInput")
    d_gin = nc.dram_tensor("gin", (128, GW), f32, kind="ExternalInput")

    d_val8 = nc.dram_tensor("val8", (128, NT * BPC * 8), f32,
                            kind="ExternalOutput")
    d_idx8 = nc.dram_tensor("idx8", (128, NT * BPC * 8), ui32,
                            kind="ExternalOutput")
    d_align = nc.dram_tensor("align", (LU, KU), f32, kind="ExternalOutput")

    with tl.TileContext(nc) as tc:
        with tc.tile_pool(name="keep", bufs=1) as keep:
            ident = keep.tile([128, 128], f32, tag="ident")
            nc.sync.dma_start(ident[:], d_ident[:])
            cst = keep.tile([128, M * NE], f32, tag="cst")
            nc.sync.dma_start(cst[:], d_cst[:])
            pscal = keep.tile([128, NT], f32, tag="pscal")
            nc.sync.dma_start(pscal[:], d_pscal[:])
            poslpT = keep.tile([KU, BPC, SPOS], f32, tag="poslpT")
            val8 = keep.tile([128, NT, BPC, 8], f32, tag="val8")
            idx8 = keep.tile([128, NT, BPC, 8], ui32, tag="idx8")

            # ---------- front-end ----------
            with (
                tc.tile_pool(name="fe", bufs=1) as fe,
                tc.tile_pool(name="feps", bufs=3, space="PSUM") as feps,
            ):
                alT = fe.tile([KU, LU], f32, tag="alT")
                ur = fe.tile([KU, DIM], f32, tag="ur")
                urT = fe.tile([DIM, KU], f32, tag="urT")
                cwT = fe.tile([DIM, 3, DIM], f32, tag="cwT")
                cb = fe.tile([DIM, 1], f32, tag="cb")
                ohu = fe.tile([LU, BPC, L + 2], f32, tag="ohu")
                ohu2 = fe.tile([LU + 1, BPC * L], f32, tag="ohu2")
                nc.sync.dma_start(alT[:], d_alT[:])
                nc.sync.dma_start(ur[:], d_ur[:])
                nc.sync.dma_start(urT[:], d_urT[:])
                nc.sync.dma_start(cwT[:], d_cwT[:])
                nc.sync.dma_start(cb[:], d_cb[:])
                nc.sync.dma_start(ohu[:], d_ohu[:])
                nc.sync.dma_start(ohu2[:], d_ohu2[:])

                # kcr = aligner @ unit_repr  (LU, DIM)
                kcr_ps = feps.tile([LU, DIM], f32, tag="fep")
                nc.tensor.matmul(kcr_ps[:], alT[:], ur[:], start=True, stop=True)
                kcr = fe.tile([LU, DIM], f32, tag="kcr")
                nc.vector.tensor_copy(kcr[:], kcr_ps[:])

                # kcrT via PE transpose
                kcrT_ps = feps.tile([DIM, LU], f32, tag="fep")
                nc.tensor.transpose(kcrT_ps[:], kcr[:], ident[:LU, :LU])
                kcrT = fe.tile([DIM, LU], f32, tag="kcrT")
                nc.vector.tensor_copy(kcrT[:], kcrT_ps[:])

                # M2 = kcr @ unit_repr.T  (LU, KU); log-softmax over KU
                m2_ps = feps.tile([LU, KU], f32, tag="fep")
                nc.tensor.matmul(m2_ps[:], kcrT[:], urT[:], start=True, stop=True)
                nmax1 = fe.tile([LU, 1], f32, tag="nmax1")
                nc.vector.tensor_reduce(nmax1[:], m2_ps[:], axis=AX.X, op=OP.max,
                                        negate=True)
                ex1 = fe.tile([LU, KU], f32, tag="ex1")
                sum1 = fe.tile([LU, 1], f32, tag="sum1")
                nc.scalar.activation(ex1[:], m2_ps[:], AF.Exp, bias=nmax1[:],
                                     scale=1.0, accum_out=sum1[:])
                lse1 = fe.tile([LU, 1], f32, tag="lse1")
                nc.scalar.activation(lse1[:], sum1[:], AF.Ln)
                clp = fe.tile([LU + 1, KU], f32, tag="clp")
                nc.vector.tensor_scalar(clp[:LU, :], m2_ps[:], nmax1[:], lse1[:],
                                        op0=OP.add, op1=OP.subtract)
                nc.sync.dma_start(clp[LU:LU + 1, :], d_c2[:])
                align_sb = fe.tile([LU, KU], f32, tag="align")
                nc.scalar.activation(align_sb[:], clp[:LU, :], AF.Exp)
                nc.sync.dma_start(d_align[:], align_sb[:])

                # embT (padded) = kcr.T @ onehotU  (DIM, BPC, L+2)
                emb_ps = feps.tile([DIM, BPC * (L + 2)], f32, tag="fep")
                nc.tensor.matmul(emb_ps[:], kcr[:],
                                 ohu[:].rearrange("p b l -> p (b l)"),
                                 start=True, stop=True)
                embT = fe.tile([DIM, BPC, L + 2], f32, tag="embT")
                nc.scalar.copy(embT[:].rearrange("p b l -> p (b l)"), emb_ps[:])

                # conv via 3 accumulating matmuls -> word_repr.T (DIM, BPC*L)
                y_ps = feps.tile([DIM, BPC * L], f32, tag="fep")
                for k in range(3):
                    nc.tensor.matmul(y_ps[:], cwT[:, k, :], embT[:, :, k:k + L],
                                     start=(k == 0), stop=(k == 2))
                wordT = fe.tile([DIM, BPC * L], f32, tag="wordT")
                nc.scalar.activation(wordT[:], y_ps[:], AF.Identity, bias=cb[:],
                                     scale=1.0)

                # ctx = word_repr @ unit_repr.T; log-softmax over KU
                ctx_ps = feps.tile([BPC * L, KU], f32, tag="fep")
                nc.tensor.matmul(ctx_ps[:], wordT[:], urT[:], start=True,
                                 stop=True)
                nmax5 = fe.tile([BPC * L, 1], f32, tag="nmax5")
                nc.vector.tensor_reduce(nmax5[:], ctx_ps[:], axis=AX.X, op=OP.max,
                                        negate=True)
                ex5 = fe.tile([BPC * L, KU], f32, tag="ex5")
                sum5 = fe.tile([BPC * L, 1], f32, tag="sum5")
                nc.scalar.activation(ex5[:], ctx_ps[:], AF.Exp, bias=nmax5[:],
                                     scale=1.0, accum_out=sum5[:])
                lse5 = fe.tile([BPC * L, 1], f32, tag="lse5")
                nc.scalar.activation(lse5[:], sum5[:], AF.Ln)
                ctx = fe.tile([BPC * L, KU], f32, tag="ctx")
                nc.vector.tensor_scalar(ctx[:], ctx_ps[:], nmax5[:], lse5[:],
                                        op0=OP.add, op1=OP.subtract)

                # clpg + 2C via augmented one-hot matmul  (BPC*L, KU)
                p6_ps = feps.tile([BPC * L, KU], f32, tag="fep")
                nc.tensor.matmul(p6_ps[:], ohu2[:], clp[:], start=True, stop=True)
                # pos_lp' = 0.1*ctx + (clpg + 2C)
                poslp = fe.tile([BPC * L, KU], f32, tag="poslp")
                nc.vector.scalar_tensor_tensor(poslp[:], ctx[:], CW, p6_ps[:],
                                               op0=OP.mult, op1=OP.add)
                # transpose -> (KU, BPC*L) -> poslpT with NEG+2C pads
                pT_ps = feps.tile([KU, BPC * L], f32, tag="fep")
                nc.tensor.transpose(pT_ps[:], poslp[:], ident[:BPC * L, :BPC * L])
                for b in range(BPC):
                    nc.scalar.copy(poslpT[:, b, 0:L], pT_ps[:, b * L:(b + 1) * L])
                nc.vector.memset(poslpT[:, :, L:SPOS], NEG + TWOC)

            # ---------- per-tile: gather subT, DP, score, top8 ----------
            with (
                tc.tile_pool(name="rot", bufs=2) as rot,
                tc.tile_pool(name="dps", bufs=2, space="PSUM") as dps,
            ):
                blk0 = 0
                goff = 0
                for t, (l, runs) in enumerate(tiles):
                    wt = l + 1
                    ohvt = rot.tile([KU, l, TP], f32, tag="ohvt", name=f"ohvt{t}")
                    nc.sync.dma_start(
                        ohvt[:].rearrange("p j v -> p (j v)"),
                        d_ohv[:, blk0 * TP:(blk0 + l) * TP])
                    blk0 += l
                    subt = rot.tile([128, BPC, SPOS, wt], f32, tag="subt",
                                    name=f"subt{t}")
                    nc.vector.memset(subt[:, :, :, 0], BIGNEG)
                    for jj in range(l):
                        psg = dps.tile([128, 2, 512], f32, tag="A",
                                       name=f"psg{t}_{jj}")
                        nc.tensor.matmul(psg[:, 0, 0:BPC * SPOS],
                                         ohvt[:, jj, :],
                                         poslpT[:].rearrange("p b s -> p (b s)"),
                                         start=True, stop=True)
                        nc.scalar.copy(
                            subt[:, :, :, jj + 1],
                            psg[:, 0, 0:BPC * SPOS].rearrange(
                                "p (b s) -> p b s", b=BPC))

                    gprev = rot.tile([128, 1 + M * wt], f32, tag="g",
                                     name=f"g{t}_0", bufs=6)
                    nc.sync.dma_start(gprev[:], d_gin[:, goff:goff + 1 + M * wt])
                    goff += 1 + M * wt
                    llt = rot.tile([128, M, NE], f32, tag="llb", name=f"llb{t}")
                    pcov = max(p0 + (-(-nr // 32)) * 32 for (p0, nr, _, _)
                               in runs)
                    for (q0, q1) in _branges(pcov, 128):
                        nc.vector.memset(llt[q0:q1, :, :], BIGNEG)

                    slabs = _slab_plan(l, mh)
                    for i in range(1, MAX_WL + 1):
                        gcur = rot.tile([128, 1 + M * wt], f32, tag="g",
                                        name=f"g{t}_{i}", bufs=6)
                        nc.vector.memset(gcur[:, 0:1], 0.0)
                        if pe_mode[t]:
                            for k0 in range(0, len(slabs), 2):
                                A = dps.tile([128, 2, 512], f32, tag="A",
                                             name=f"A{t}_{i}_{k0}")
                                pair = slabs[k0:k0 + 2]
                                for c_, (m0, m1) in enumerate(pair):
                                    ln = (m1 - m0) * wt
                                    b = 0 if m1 <= mh[0] else 1
                                    s0 = m0 - (0 if b == 0 else mh[0])
                                    nc.tensor.matmul(
                                        A[:, c_, 0:ln], ident[:],
                                        subt[:, b, i - 1 + s0:
                                             i - 1 + s0 + (m1 - m0), :],
                                        start=True, stop=False)
                                    nc.tensor.matmul(
                                        A[:, c_, 0:ln], ident[:],
                                        gprev[:, m0 * wt:m1 * wt],
                                        start=False, stop=True)
                                for c_, (m0, m1) in enumerate(pair):
                                    ln = (m1 - m0) * wt
                                    nc.vector.tensor_tensor_scan(
                                        gcur[:, 1 + m0 * wt:1 + m1 * wt],
                                        gprev[:, 1 + m0 * wt:1 + m1 * wt],
                                        A[:, c_, 0:ln], 0.0,
                                        op0=OP.max, op1=OP.max)
                        else:
                            asb = rot.tile([128, 1 + M * wt], f32, tag="asb",
                                           name=f"asb{t}_{i}", bufs=2)
                            off = 0
                            for b in range(BPC):
                                w = mh[b]
                                nc.vector.tensor_add(
                                    asb[:, 1 + off * wt:1 + (off + w) * wt],
                                    gprev[:, off * wt:(off + w) * wt],
                                    subt[:, b, i - 1:i - 1 + w, :])
                                off += w
                            nc.vector.tensor_tensor_scan(
                                gcur[:, 1:1 + M * wt],
                                gprev[:, 1:1 + M * wt],
                                asb[:, 1:1 + M * wt], 0.0,
                                op0=OP.max, op1=OP.max)
                        if i >= MIN_WL:
                            gv = gcur[:, 1:].rearrange("p (m w) -> p m w", w=wt)
                            for (p0, nr, lr, _) in runs:
                                p1 = min(128, p0 + (-(-nr // 32)) * 32)
                                for (q0, q1) in _branges(p0, p1):
                                    nc.scalar.copy(llt[q0:q1, :, i - MIN_WL],
                                                   gv[q0:q1, :, lr])
                        gprev = gcur

                    flat = llt[:].rearrange("p m e -> p (m e)")
                    nc.vector.scalar_tensor_tensor(flat, flat, pscal[:, t:t + 1],
                                                   cst[:], op0=OP.add, op1=OP.add)
                    off = 0
                    for b in range(BPC):
                        sl = flat[:, off * NE:(off + mh[b]) * NE]
                        nc.vector.max(val8[:, t, b, :], sl)
                        nc.vector.max_index(idx8[:, t, b, :], val8[:, t, b, :], sl)
                        off += mh[b]

            nc.sync.dma_start(d_val8[:],
                              val8[:].rearrange("p t b k -> p (t b k)"))
            nc.sync.dma_start(d_idx8[:],
                              idx8[:].rearrange("p t b k -> p (t b k)"))

    nc.compile()
    return nc


# --------------------------------------------------------------------------
# host-side input construction
# --------------------------------------------------------------------------
def _host_inputs(inp, order, vls, tiles, pairs, mh):
    uid = inp["unit_id_seqs"].astype(np.int64)
    lengths = inp["lengths"].astype(np.int64)
    vid_s = inp["vocab_ids"].astype(np.int64)[order]
    NOH = sum(l for l, _ in tiles)
    NT = len(tiles)
    M = sum(mh)

    rep = {}
    rep["alT"] = np.ascontiguousarray(inp["aligner_weight"].T.astype(np.float32))
    rep["ur"] = inp["unit_repr"].astype(np.float32)
    rep["urT"] = np.ascontiguousarray(inp["unit_repr"].T.astype(np.float32))
    rep["cwT"] = np.ascontiguousarray(
        inp["conv_w"].astype(np.float32).transpose(1, 2, 0))
    rep["cb"] = inp["conv_b"].astype(np.float32).reshape(DIM, 1)
    rep["c2"] = np.full((1, KU), TWOC, np.float32)
    rep["ident"] = np.eye(128, dtype=np.float32)

    ohv = np.zeros((KU, NOH * TP), np.float32)
    pscal = np.zeros((128, NT), np.float32)
    blk = 0
    for t, (l, runs) in enumerate(tiles):
        for jj in range(l):
            for (p0, nr, lr, base) in runs:
                if jj < lr:
                    cols = blk * TP + p0 + np.arange(nr)
                    ohv[vid_s[base:base + nr, jj], cols] = 1.0
            blk += 1
        for (p0, nr, lr, base) in runs:
            p1 = min(128, p0 + (-(-nr // 32)) * 32)
            pscal[p0:p1, t] = -C * lr
    rep["ohv"] = ohv
    rep["pscal"] = pscal

    GW = sum(1 + M * (l + 1) for l, _ in tiles)
    gin = np.zeros((1, GW), np.float32)
    off = 0
    for l, _ in tiles:
        wt = l + 1
        gin[0, off + 1:off + 1 + M * wt] = (
            K_OFF * (np.arange(M * wt) // wt)).astype(np.float32)
        off += 1 + M * wt
    rep["gin"] = np.broadcast_to(gin, (128, GW)).copy()

    in_maps = []
    for c in range(N_CORES):
        bg = pairs[c]
        m_ = dict(rep)
        ohu = np.zeros((LU, BPC, L + 2), np.float32)
        for b in range(BPC):
            ohu[uid[bg[b]], b, 1 + np.arange(L)] = 1.0
        m_["ohu"] = ohu
        ohu2 = np.zeros((LU + 1, BPC * L), np.float32)
        for b in range(BPC):
            ohu2[uid[bg[b]], b * L + np.arange(L)] = 1.0
        ohu2[LU, :] = 1.0
        m_["ohu2"] = ohu2
        cst = np.full((M, NE), BIGNEG, np.float32)
        off = 0
        wl = MIN_WL + np.arange(NE)
        for b in range(BPC):
            lb = lengths[bg[b]]
            s_ = np.arange(mh[b])[:, None]
            blkv = (-K_OFF * (off + s_) - C * wl[None, :]
                    + (lb - wl)[None, :] * LOGU
                    + np.where(s_ + wl[None, :] > lb, NEG, 0.0))
            cst[off:off + mh[b], :] = blkv
            off += mh[b]
        cst = cst.reshape(M * NE)
        m_["cst"] = np.broadcast_to(cst, (128, M * NE)).copy()
        in_maps.append(m_)
    return in_maps


# --------------------------------------------------------------------------
# exact float64 re-scoring of candidates (reference-equivalent math)
# --------------------------------------------------------------------------
def _host_pos_lp(inp):
    aw = inp["aligner_weight"].astype(np.float64)
    ur = inp["unit_repr"].astype(np.float64)
    cw = inp["conv_w"].astype(np.float64)
    cb = inp["conv_b"].astype(np.float64)
    uid = inp["unit_id_seqs"].astype(np.int64)

    def lsm(x):
        mx = x.max(-1, keepdims=True)
        return x - mx - np.log(np.exp(x - mx).sum(-1, keepdims=True))

    kcr = aw @ ur
    emb = kcr[uid]                                  # (B, L, D)
    xp = np.pad(emb, ((0, 0), (1, 1), (0, 0)))
    y = np.zeros((B, L, DIM))
    for k in range(3):
        y += xp[:, k:k + L, :] @ cw[:, :, k].T
    word = y + cb
    clp = lsm(kcr @ ur.T)
    ctx = lsm(word @ ur.T)
    return clp[uid] + CW * ctx, clp                 # (B, L, KU), (LU, KU)


def _rescore(pos_lp, inp, cands):
    vid = inp["vocab_ids"].astype(np.int64)
    vl = inp["vocab_lengths"].astype(np.int64)
    lengths = inp["lengths"].astype(np.int64)
    nc_ = len(cands)
    bs = np.array([c[0] for c in cands])
    ss = np.array([c[1] for c in cands])
    es = np.array([c[2] for c in cands])
    vs = np.array([c[3] for c in cands])
    plp_pad = np.concatenate(
        [pos_lp, np.full((B, MAX_WL, KU), NEG)], axis=1)   # (B, L+10, KU)
    # sub[c, i-1, j] = pos_lp[b, s+i-1, vid[v, j]]
    sub = plp_pad[bs[:, None, None], (ss[:, None] + np.arange(MAX_WL))[:, :, None],
                  vid[vs][:, None, :]]                      # (nc, 10, J)
    prev = np.broadcast_to(-C * np.arange(J + 1), (nc_, J + 1)).copy()
    i_n = MIN_WL + es
    ll = np.zeros(nc_)
    for i in range(1, MAX_WL + 1):
        row = np.empty((nc_, J + 1))
        row[:, 0] = -C * i
        for j in range(1, J + 1):
            row[:, j] = np.maximum(prev[:, j - 1] + sub[:, i - 1, j - 1],
                                   np.maximum(prev[:, j] - C,
                                              row[:, j - 1] - C))
        prev = row
        sel = i_n == i
        if sel.any():
            ll[sel] = prev[sel, vl[vs[sel]]]
    sc = ll + (lengths[bs] - i_n) * LOGU
    sc = np.where(ss + i_n > lengths[bs], sc + NEG, sc)
    return sc


def _vmap(order, tiles):
    """(t, p) -> global vocab id, or -1 for pad partitions."""
    NT = len(tiles)
    vm = np.full((NT, 128), -1, np.int64)
    for t, (l, runs) in enumerate(tiles):
        for (p0, nr, lr, base) in runs:
            vm[t, p0:p0 + nr] = order[base:base + nr]
    return vm


def _decode(vm, tiles, val, idx, b, bl, topk=48):
    """val/idx: (128, NT, BPC, 8) arrays for one core; candidates for row b."""
    NT = len(tiles)
    vals = np.where(vm.T[:, :, None] >= 0, val[:, :, bl, :], -np.inf)
    flat = np.argsort(-vals, axis=None, kind="stable")[:topk]
    cands = []
    for f in flat:
        p, t, k = np.unravel_index(f, vals.shape)
        li = int(idx[p, t, bl, k])
        s, e = li // NE, li % NE
        cands.append((b, int(s), int(e), int(vm[t, p])))
    return cands


def kernel(**inputs):
    from concourse.bass_utils import run_bass_kernel_spmd

    inp = {k: np.asarray(v) for k, v in inputs.items()}
    order, vls, tiles = _vocab_meta(inp["vocab_lengths"].astype(np.int64))
    pairs, mh = _pair_batches(inp["lengths"].astype(np.int64))

    key = (vls.tobytes(), tuple(mh), PE_FRAC)
    if key not in _CACHE:
        _CACHE[key] = _build_program(tiles, mh)
    nc = _CACHE[key]

    in_maps = _host_inputs(inp, order, vls, tiles, pairs, mh)
    prof = os.environ.get("KERNEL_PROFILE") == "1"
    try:
        res = run_bass_kernel_spmd(nc, in_maps, core_ids=list(range(N_CORES)),
                                   trace=prof)
    except Exception:
        if not prof:
            raise
        res = run_bass_kernel_spmd(nc, in_maps, core_ids=list(range(N_CORES)),
                                   trace=False)
    kernel.last_exec_ns = res.exec_time_ns

    NT = len(tiles)
    vm = _vmap(order, tiles)
    pos_lp, _ = _host_pos_lp(inp)
    start = np.zeros(B, np.int32)
    end = np.zeros(B, np.int32)
    bll = np.zeros(B, np.float32)
    bvoc = np.zeros(B, np.int32)
    for c in range(N_CORES):
        val = res.results[c]["val8"].reshape(128, NT, BPC, 8)
        idx = res.results[c]["idx8"].reshape(128, NT, BPC, 8).astype(np.int64)
        for bl in range(BPC):
            b = pairs[c][bl]
            cands = _decode(vm, tiles, val, idx, b, bl)
            sc = _rescore(pos_lp, inp, cands)
            # replicate reference argmax tie-breaking: fp32 scores, first
            # flat index (s, e, v) wins among ties
            sc32 = sc.astype(np.float32)
            fidx = np.array([s * (NE * V) + e * V + v
                             for (_, s, e, v) in cands], np.int64)
            best = sc32.max()
            pool = np.where(sc32 == best)[0]
            w = int(pool[np.argmin(fidx[pool])])
            _, sw, ew, vw = cands[w]
            start[b] = sw
            end[b] = sw + MIN_WL + ew - 1
            bll[b] = np.float32(sc[w])
            bvoc[b] = vw
    alignment = res.results[0]["align"].astype(np.float32)
    return start, end, bll, bvoc, alignment


kernel.last_exec_ns = None
